# revision 57
# baseline (speedup 1.0000x reference)
"""Trainium2 Bass kernel for DescriptorMatchLoss (retrieval_knn).

Reference computation (per batch-pair grid [B,B]):
    d2[i,j,n,m] = ||denorm(pts_src[i,n]) - denorm(pts_dst[i,j,m])||^2
    mask        = d2 <= RADIUS^2
    cos[i,j,n,m] = <fhat[j,n], fhat[i,m]>   (fhat = row-normalized features)
    loss = sum(mask * (1 - cos)) / max(sum(mask), 1)

Block-sparse device strategy (8 cores, 2 (i,j) pairs per core):
  * Rows (n) of each pair are sorted by the x coordinate of the source
    point, columns (m) by the x coordinate of the destination point
    (host-side permutations).  A mask block can then only be nonzero when
    the 128-row tile's x-range and the 256-column chunk's x-range are
    within RADIUS of each other — for uniform points that keeps only
    ~23% of the [N,N] grid (contiguous n-tile "runs" per column chunk).
    Skipped blocks are *exactly* zero, so the result is unchanged.
  * Runs are unioned across the 8 cores so one SPMD program serves all.
  * Per active unit u = (slot, mc): a [128, L*256] z-tile (z = R^2 - d2)
    via L K=14 bf16 matmuls (hi/lo-split coordinates, exact in fp32),
    then ONE mask op covering the whole run:
      - ACT units: Sign(z) in {-1,0,1} fp8, accum_out = sum (count via
        affine fix on host)
      - DVE units: z >= 0 in {1,0} fp8, accum_out = count
  * G''[m,d] = sum_n mask[n,m] * fj8[n,d] with the mask as the matmul
    stationary operand (fp8 DoubleRow over n-tile pairs) and fj8 moving,
    accumulated over the run in a [128, 2*256] PSUM tile (m-chunk halves
    side by side).
  * Extraction: ONE fused scalar_tensor_tensor per unit:
      accum_out = sum_d,m G''[m,d] * fi8[m,d]   (internal fp32 reduce)
  * Host combines: per-ACT-unit affine corrections (exact fp64 from the
    fp8-quantized operands), then loss = (A - B) / max(A, 1).

kernel(**inputs) takes FULL inputs, shards pairs across 8 cores, returns
the scalar loss (fp32).  The bass program is specialized on the sparsity
structure of the actual inputs (recompiled if the geometry changes).
"""

import sys

for _p in ("/opt/pypackages", "/opt/trn_rl_repo"):
    if _p not in sys.path:
        sys.path.insert(0, _p)

import numpy as np
import ml_dtypes

BF16 = ml_dtypes.bfloat16
FP8 = ml_dtypes.float8_e4m3

# Problem constants (hardcoded per contract).
B, N, D = 4, 2048, 256
HEIGHT, WIDTH = 480, 640
RADIUS = 8.0
RADIUS2 = RADIUS * RADIUS
RUN_MARGIN = 0.02          # px slack when deciding block runs
N_CORES = 8
SLOTS = 2                  # (i,j) pairs per core
P = 128                    # partitions
NT = N // P                # 16 n-tiles
MC = 256                   # m-chunk width (mask/d2 block width)
NMC = N // MC              # 8 column chunks per pair
NU = SLOTS * NMC           # 16 units per core
KGEO = 14                  # geometry contraction rows
MAX_L = 4                  # d2 run-tile capacity (n-tiles per PSUM tile)

SLOT_MAJOR = False   # emission order groups slot 0 before slot 1
TAIL_DVE = True      # last emitted unit uses the DVE mask engine
OUT_CHUNK = 16       # units per chunked result DMA

_CACHE = {}
LAST = None  # BassKernelResults of the most recent run (for test harness)


# ---------------------------------------------------------------- host prep

def _split2(x):
    hi = x.astype(BF16)
    lo = (x - hi.astype(np.float64)).astype(BF16)
    return hi, lo


def _split3(x):
    hi = x.astype(BF16)
    r = x - hi.astype(np.float64)
    mid = r.astype(BF16)
    lo = (r - mid.astype(np.float64)).astype(BF16)
    return hi, mid, lo


def _geo_rows(psc, pdc):
    """14-row geometry operands such that
    z[n, m] = RADIUS2 - ||p_n - q_m||^2 = sum_k L[k, n] * R[k, m],
    exact to ~fp32 via bf16 hi/lo splits.  psc: [N,2] fp64 centered
    source coords, pdc: [N,2] fp64 centered dest coords."""
    phx, plx = _split2(psc[:, 0])
    phy, ply = _split2(psc[:, 1])
    qhx, qlx = _split2(pdc[:, 0])
    qhy, qly = _split2(pdc[:, 1])

    sh, sm, sl = _split3(
        RADIUS2
        - (
            (phx.astype(np.float64) + plx.astype(np.float64)) ** 2
            + (phy.astype(np.float64) + ply.astype(np.float64)) ** 2
        )
    )
    tq = (
        (qhx.astype(np.float64) + qlx.astype(np.float64)) ** 2
        + (qhy.astype(np.float64) + qly.astype(np.float64)) ** 2
    )
    th, tm, tl = _split3(tq)

    ones = np.ones((N,), dtype=BF16)
    p2hx = (2.0 * phx.astype(np.float64)).astype(BF16)
    p2lx = (2.0 * plx.astype(np.float64)).astype(BF16)
    p2hy = (2.0 * phy.astype(np.float64)).astype(BF16)
    p2ly = (2.0 * ply.astype(np.float64)).astype(BF16)
    geoL = np.stack(
        [p2hx, p2hx, p2lx, p2lx, p2hy, p2hy, p2ly, p2ly,
         sh, sm, sl, -ones, -ones, -ones], axis=0)          # [14, N]
    geoR = np.stack(
        [qhx, qlx, qhx, qlx, qhy, qly, qhy, qly,
         ones, ones, ones, th, tm, tl], axis=0)             # [14, N]
    return geoL, geoR


def _host_prep(features, pts_src, pts_dst, height, width):
    height = int(height)
    width = int(width)
    scale32 = np.array([(width - 1) * 0.5, (height - 1) * 0.5],
                       dtype=np.float32)

    # Match the reference's fp32 denorm rounding, then center.
    ps32 = (pts_src.astype(np.float32) + np.float32(1.0)) * scale32
    pd32 = (pts_dst.astype(np.float32) + np.float32(1.0)) * scale32
    psc = ps32.astype(np.float64) - scale32.astype(np.float64)  # [B,N,2]
    pdc = pd32.astype(np.float64) - scale32.astype(np.float64)  # [B,B,N,2]

    f64 = features.astype(np.float64)
    norms = np.sqrt((f64 * f64).sum(-1, keepdims=True))
    fhat8 = (f64 / norms).astype(BF16).astype(FP8)              # [B,N,D]

    # Per-core pair assignment: core c -> i = c//2, j_s = (2c+s) % B.
    cores = []
    for c in range(N_CORES):
        i = (2 * c) // B
        js = [(2 * c + s) % B for s in range(SLOTS)]
        rho = np.argsort(psc[i, :, 0], kind="stable")           # row perm
        sigs = [np.argsort(pdc[i, j, :, 0], kind="stable") for j in js]
        cores.append((i, js, rho, sigs))

    # Block runs per (slot, mc), unioned across cores so the single SPMD
    # program covers every core's sparsity pattern exactly.
    runs = np.zeros((SLOTS, NMC, 2), dtype=np.int64)  # (lo, hi)
    runs[:, :, 0] = NT
    runs[:, :, 1] = 0
    for c in range(N_CORES):
        i, js, rho, sigs = cores[c]
        rsx = psc[i, rho, 0]
        rlo = rsx[0::P]
        rhi = rsx[P - 1::P]
        for s in range(SLOTS):
            csx = pdc[i, js[s], sigs[s], 0]
            for mc in range(NMC):
                clo = csx[mc * MC]
                chi = csx[mc * MC + MC - 1]
                act = [k for k in range(NT)
                       if not (rlo[k] > chi + RADIUS + RUN_MARGIN
                               or rhi[k] < clo - RADIUS - RUN_MARGIN)]
                runs[s, mc, 0] = min(runs[s, mc, 0], act[0])
                runs[s, mc, 1] = max(runs[s, mc, 1], act[-1] + 1)
    run_list = []
    for s in range(SLOTS):
        for mc in range(NMC):
            lo, hi = int(runs[s, mc, 0]), int(runs[s, mc, 1])
            assert 0 <= lo < hi <= NT
            run_list.append((s, mc, lo, hi - lo))

    # Engine assignment per unit: greedy balance of ACT vs DVE given the
    # cost model (DVE also runs the 16 fused extraction ops).
    def act_cost(L):
        return L * MC * 0.8333 + 185.0 + 187.0

    def dve_cost(L):
        return L * MC * 1.0417 + 125.0

    stt_cost = NU * (512 * 1.0417 + 125.0)
    loads = {"act": 0.0, "dve": stt_cost}
    order = sorted(range(NU), key=lambda u: -run_list[u][3])
    conv = [""] * NU
    for u in order:
        L = run_list[u][3]
        if loads["act"] + act_cost(L) <= loads["dve"] + dve_cost(L):
            conv[u] = "act"
            loads["act"] += act_cost(L)
        else:
            conv[u] = "dve"
            loads["dve"] += dve_cost(L)

    # Emission order: ACT/DVE-interleaved (the z-PSUM pool keeps mask
    # production near emission order, so clustering either engine's units
    # would idle the other engine at the start).  SLOT_MAJOR additionally
    # groups slot 0 first so late feature DMAs overlap slot-0 compute;
    # TAIL_DVE ends with a DVE-masked unit so the final extraction follows
    # the final mask immediately.
    def interleave(units):
        acts = [u for u in units if conv[u] == "act"]
        dves = [u for u in units if conv[u] == "dve"]
        return sorted(
            units,
            key=lambda u: ((acts.index(u) + 0.5) / max(len(acts), 1)
                           if conv[u] == "act"
                           else (dves.index(u) + 0.5) / max(len(dves), 1)))

    if SLOT_MAJOR:
        emit_order = []
        for s in range(SLOTS):
            emit_order += interleave(
                [u for u in range(NU) if run_list[u][0] == s])
    else:
        emit_order = interleave(list(range(NU)))
    if TAIL_DVE:
        tail = [u for u in emit_order if conv[u] == "dve"][-1:]
        emit_order = [u for u in emit_order if u not in tail] + tail
    run_list = [run_list[u] for u in emit_order]
    conv = [conv[u] for u in emit_order]

    # z-tile capacity adapts to the longest run; PSUM pool depth shrinks
    # if the tiles outgrow the 8-bank budget (never triggers for the
    # uniform-points regime where L <= 4).
    max_l = max(r[3] for r in run_list)
    spec = (tuple(run_list), tuple(conv), max(MAX_L, max_l))

    # Per-core device inputs + correction data.
    in_maps = []
    combine = []
    for c in range(N_CORES):
        i, js, rho, sigs = cores[c]
        geoL, _ = _geo_rows(psc[i, rho], pdc[i, js[0], sigs[0]])
        geoR = np.zeros((SLOTS, KGEO, N), dtype=BF16)
        fj8 = np.zeros((SLOTS, NT, P, D), dtype=FP8)
        fi8 = np.zeros((SLOTS, NMC, 2, P, D), dtype=FP8)
        for s in range(SLOTS):
            _, gR = _geo_rows(psc[i, rho], pdc[i, js[s], sigs[s]])
            geoR[s] = gR
            fj8[s] = fhat8[js[s]][rho].reshape(NT, P, D)
            fi8[s] = fhat8[i][sigs[s]].reshape(NMC, 2, P, D)
        geo = np.concatenate(
            [geoL.astype(BF16)[:, None, :], geoR.transpose(1, 0, 2)], axis=1)
        in_maps.append({
            # DRAM layouts mirror the SBUF tiles exactly so every input DMA
            # is a contiguous >=512B-burst copy (full DMA bus rate).
            "fj8": np.ascontiguousarray(fj8.transpose(2, 0, 1, 3)),
            "fi8": np.ascontiguousarray(fi8.transpose(3, 0, 1, 2, 4)),
            "geo": np.ascontiguousarray(geo),
        })
        combine.append((fj8.astype(np.float64), fi8.astype(np.float64)))
    return in_maps, spec, run_list, conv, combine


# ---------------------------------------------------------------- bass build

def _build_bass(spec, reps=1):
    import concourse.bass as bass
    import concourse.mybir as mybir
    import concourse.tile as tile

    run_list, conv, max_l = spec
    z_banks = (max_l * MC * 4 + 2047) // 2048
    z_bufs = max(2, min(3, (8 - 2) // z_banks))
    nc = bass.Bass(trn_type="TRN2", target_bir_lowering=False, debug=False)
    f32 = mybir.dt.float32
    bf16 = mybir.dt.bfloat16
    fp8 = mybir.dt.float8e4

    fj_d = nc.dram_tensor("fj8", [P, SLOTS, NT, D], fp8, kind="ExternalInput")
    fi_d = nc.dram_tensor("fi8", [P, SLOTS, NMC, 2, D], fp8,
                          kind="ExternalInput")
    # geo[:, 0, :] = geoL (shared); geo[:, 1+s, :] = geoR of slot s
    geo_d = nc.dram_tensor("geo", [KGEO, 1 + SLOTS, N], bf16,
                           kind="ExternalInput")
    out_d = nc.dram_tensor("out", [P, 2 * NU], f32, kind="ExternalOutput")

    with tile.TileContext(nc) as tc:
        with (
            tc.tile_pool(name="feat", bufs=1) as feat_pool,
            tc.tile_pool(name="geo", bufs=1) as geo_pool,
            tc.tile_pool(name="acc", bufs=1) as acc_pool,
            tc.tile_pool(name="mask", bufs=4) as mask_pool,
            tc.tile_pool(name="trash", bufs=2) as trash_pool,
            tc.tile_pool(name="psum_z", bufs=z_bufs, space="PSUM") as z_pool,
            tc.tile_pool(name="psum_g", bufs=2, space="PSUM") as g_pool,
        ):
            fj_sb = feat_pool.tile([P, SLOTS, NT, D], fp8)
            fi_sb = feat_pool.tile([P, SLOTS, NMC, 2, D], fp8)
            geo_sb = geo_pool.tile([P, 1 + SLOTS, N], bf16)
            geoL_sb = geo_sb[:, 0, :]
            geoR_sb = geo_sb[:, 1:, :]
            acc = acc_pool.tile([P, 2, NU], f32)  # [ext | cnt] interleaved
            ext_acc = acc[:, 0, :]
            cnt_acc = acc[:, 1, :]

            nc.sync.dma_start(out=geo_sb[0:KGEO, :, :], in_=geo_d[:])
            for s in range(SLOTS):
                nc.sync.dma_start(out=fj_sb[:, s, :, :], in_=fj_d[:, s])
                nc.sync.dma_start(out=fi_sb[:, s, :, :, :], in_=fi_d[:, s])

            def emit_z(u):
                s, mc, lo, L = run_list[u]
                z_t = z_pool.tile([P, max_l * MC], f32, tag="z")
                for k in range(L):
                    nt = lo + k
                    nc.tensor.matmul(
                        z_t[:, k * MC:(k + 1) * MC],
                        geoL_sb[0:KGEO, nt * P:(nt + 1) * P],
                        geoR_sb[0:KGEO, s, mc * MC:(mc + 1) * MC],
                        start=True, stop=True)
                return z_t

            def emit_mask(u, z_t):
                s, mc, lo, L = run_list[u]
                mask_t = mask_pool.tile([P, max_l, MC], fp8, tag="mask")
                z_in = z_t[:, 0:L * MC].rearrange("p (l m) -> p l m", l=L)
                if conv[u] == "act":
                    nc.scalar.activation(
                        mask_t[:, 0:L, :], z_in,
                        mybir.ActivationFunctionType.Sign,
                        accum_out=cnt_acc[:, u:u + 1])
                else:
                    nc.vector.tensor_scalar(
                        out=mask_t[:, 0:L, :], in0=z_in,
                        scalar1=0.0, scalar2=0.0,
                        op0=mybir.AluOpType.is_ge, op1=mybir.AluOpType.add,
                        accum_out=cnt_acc[:, u:u + 1])
                return mask_t

            def emit_g(u, mask_t, g_t):
                s, mc, lo, L = run_list[u]
                npairs = L // 2
                for h in range(2):
                    col = h * D
                    for si in range(npairs):
                        nc.tensor.matmul(
                            g_t[:, col:col + D],
                            mask_t[:, 2 * si:2 * si + 2,
                                   h * P:(h + 1) * P],
                            fj_sb[:, s, lo + 2 * si:lo + 2 * si + 2, :],
                            start=(si == 0), stop=(si == npairs - 1 and
                                                   L % 2 == 0),
                            perf_mode=mybir.MatmulPerfMode.DoubleRow)
                    if L % 2 == 1:
                        nc.tensor.matmul(
                            g_t[:, col:col + D],
                            mask_t[:, L - 1, h * P:(h + 1) * P],
                            fj_sb[:, s, lo + L - 1, :],
                            start=(npairs == 0), stop=True)

            def emit_ext(u, g_t):
                s, mc, lo, L = run_list[u]
                tr = trash_pool.tile([P, 2 * D], fp8, tag="tr")
                nc.vector.scalar_tensor_tensor(
                    out=tr[:], in0=g_t[:, 0:2 * D], scalar=0.0,
                    in1=fi_sb[:, s, mc, :, :].rearrange("p h d -> p (h d)"),
                    op0=mybir.AluOpType.add, op1=mybir.AluOpType.mult,
                    accum_out=ext_acc[:, u:u + 1])

            def emit_body():
                stage = [None] * NU  # mask_t handoff
                for u in range(NU + 1):
                    if u < NU:
                        z_t = emit_z(u)
                        stage[u] = emit_mask(u, z_t)
                    if 1 <= u:
                        v = u - 1
                        g_t = g_pool.tile([P, 2 * D], f32, tag="g")
                        emit_g(v, stage[v], g_t)
                        emit_ext(v, g_t)
                        # Chunked result DMAs: each waits on only a few
                        # units' accum writes and overlaps later compute.
                        if v % OUT_CHUNK == OUT_CHUNK - 1:
                            lo = v - OUT_CHUNK + 1
                            nc.sync.dma_start(
                                out=out_d[:].rearrange(
                                    "p (a u) -> p a u", a=2)[:, :, lo:v + 1],
                                in_=acc[:, :, lo:v + 1])

            if reps == 1:
                emit_body()
            else:
                with tc.For_i(0, reps, 1):
                    emit_body()

    _split_multi_waits(nc)
    return nc


def _split_multi_waits(nc):
    """Walrus rejects >1 sync-wait on compute/DMA instruction encodings.
    Hoist all but one wait onto standalone InstEventSemaphore instructions
    immediately before the instruction on the same engine queue."""
    import concourse.mybir as mybir

    n_split = 0
    for fn in nc.m.functions:
        for bb in fn.blocks:
            new_list = []
            for inst in bb.instructions:
                si = inst.sync_info
                if (
                    si is not None
                    and si.on_wait
                    and len(si.on_wait) > 1
                    and not isinstance(inst, mybir.InstEventSemaphore)
                ):
                    waits = list(si.on_wait)
                    for k, w in enumerate(waits[:-1]):
                        n_split += 1
                        new_list.append(
                            mybir.InstEventSemaphore(
                                name=f"{inst.name}-hw{k}",
                                engine=inst.engine,
                                ins=[], outs=[],
                                sync_info=mybir.SyncInfo(
                                    on_wait=[w], on_update=[]),
                            ))
                    inst.sync_info = mybir.SyncInfo(
                        on_wait=[waits[-1]],
                        on_update=list(si.on_update or []))
                new_list.append(inst)
            bb.instructions[:] = new_list
    return n_split


def _get_bass(spec, reps=1):
    key = (spec, reps)
    if key not in _CACHE:
        _CACHE[key] = _build_bass(spec, reps=reps)
    return _CACHE[key]


# ---------------------------------------------------------------- combine

def _combine(results, run_list, conv, combine):
    A_tot = 0.0
    B_tot = 0.0
    for c in range(N_CORES):
        out = results[c]["out"].astype(np.float64)
        fj64, fi64 = combine[c]
        r = out[:, 0:NU].sum(axis=0)       # ext accums
        a = out[:, NU:2 * NU].sum(axis=0)  # count accums
        for u in range(NU):
            s, mc, lo, L = run_list[u]
            if conv[u] == "act":
                area = L * P * MC
                A_tot += 0.5 * (a[u] + area)
                S = fj64[s, lo:lo + L].reshape(L * P, D).sum(axis=0)
                FS = fi64[s, mc].reshape(2 * P, D).sum(axis=0)
                C = float(np.dot(S, FS))
                B_tot += 0.5 * (r[u] + C)
            else:
                A_tot += a[u]
                B_tot += r[u]
    return A_tot, B_tot


def kernel(features, pts_src, pts_dst, invis_idx, height, width):
    global LAST
    del invis_idx  # unused by the reference computation

    features = np.asarray(features)
    pts_src = np.asarray(pts_src)
    pts_dst = np.asarray(pts_dst)

    in_maps, spec, run_list, conv, combine = _host_prep(
        features, pts_src, pts_dst, height, width)

    from concourse.bass_utils import run_bass_kernel_spmd

    nc = _get_bass(spec)
    LAST = run_bass_kernel_spmd(nc, in_maps, core_ids=list(range(N_CORES)))

    A_tot, B_tot = _combine(LAST.results, run_list, conv, combine)
    loss = (A_tot - B_tot) / max(A_tot, 1.0)
    return np.float32(loss)
